# revision 1
# baseline (speedup 1.0000x reference)
"""Multi-head causal attention (b=4, n=2048, d=1024, h=16) on 8 trn2 cores.

Sharding: data-parallel over batch (4) x tensor-parallel over heads (2 groups
of 8 heads).  Core c handles batch c//2, heads 8*(c%2)..8*(c%2)+8.

Per-core dataflow (all matmuls in float32r: full PE rate, ~2e-4 rel err):
  xs[n] [128,8,512]   = x[b].T strip           (streamed per 512-col strip)
  w{q,k,v}T [1024,512] = W.T[:, group]         (host-prepared)
  KT [512,2048]       = wkT.T @ xT             (K transposed: [head*dim, n])
  Vp [2048, 8*65]     = xT.T @ wvT (+ ones col)
  QTs [512,512]       = wqT.T @ xs[n]          (per strip)
  per (head-pair, strip):
    S^T blocks [128 nk, 512 nq] = KT_h_blk.T @ QTs_h  (K=64 contraction;
      the pair's QK matmuls alternate PE row groups 0-63/64-127 so they
      stream concurrently; 2 blocks per PSUM chunk -> one exp(S/8) each)
    causal: 0/1 triangular multiply on the diagonal 128x128 sub-block (DVE,
      SBUF); QK and PV skip fully-masked column ranges
    [O^T; sums] accumulated in PSUM = [V|1]_blk.T @ P^T_blk over k blocks
    normalize on device: O^T * bcast(1/sums); the partition-broadcast runs
    on GPSIMD (raw sbuf tensors), then -> DRAM (Pool SWDGE)
  host gather: out[b, :, group] = outT.T

Each strip's projection groups (KT/Vp/QTs of that strip) are woven into its
own attention emission - one group per exp chunk - so the PE stays fed while
ACT (the attention bottleneck) drains the exp queue; per pair t only
[KT m=t, QT m=t] must precede it, and the new Vp blocks are woven at
2/chunk during pair 0 ahead of the diagonal PV matmuls that read them.
"""

import numpy as np

import concourse.bacc as bacc
import concourse.mybir as mybir
import concourse.tile as tile
from concourse import bass_utils
from concourse.bass_interp import get_hw_module

N_CORES = 8
B, N, D = 4, 2048, 1024
HEADS = 16
HPC = 8            # heads per core
HD = 64            # head dim
GD = HPC * HD      # 512 weight columns per core
KC = D // 128      # 8 contraction chunks of in_dim
NB = N // 128      # 16 key blocks
NSTRIP = N // 512  # 4 query strips
CH = 2             # S^T key-blocks per PSUM chunk / exp call
PSS_BUFS = 2       # PSUM: 2*2 banks S chunks + 1 proj + 2 PV accum = 7 of 8

f32 = mybir.dt.float32
f32r = mybir.dt.float32r
EXP = mybir.ActivationFunctionType.Exp


def build_program():
    nc = bacc.Bacc("TRN2", target_bir_lowering=False, debug=False,
                   num_devices=N_CORES)
    xT = nc.dram_tensor("xT", [D, N], f32r, kind="ExternalInput").ap()
    wqT = nc.dram_tensor("wqT", [D, GD], f32r, kind="ExternalInput").ap()
    wkT = nc.dram_tensor("wkT", [D, GD], f32r, kind="ExternalInput").ap()
    wvT = nc.dram_tensor("wvT", [D, GD], f32r, kind="ExternalInput").ap()
    tri01 = nc.dram_tensor("tri01", [128, 128], f32, kind="ExternalInput").ap()
    outT = nc.dram_tensor("outT", [GD, N], f32, kind="ExternalOutput").ap()
    # raw (non-pool) sbuf tensors: partition_broadcast needs concrete APs
    rec_raw = [nc.alloc_sbuf_tensor(f"rec_raw{i}", [1, 512], f32).ap()
               for i in range(2)]
    rb_raw = [nc.alloc_sbuf_tensor(f"rb_raw{i}", [64, 512], f32).ap()
              for i in range(2)]

    with tile.TileContext(nc) as tc:
        with (
            tc.tile_pool(name="xs", bufs=2) as xs_pool,
            tc.tile_pool(name="w", bufs=1) as w_pool,
            tc.tile_pool(name="big", bufs=1) as big_pool,
            tc.tile_pool(name="qt", bufs=3) as qt_pool,
            tc.tile_pool(name="pt", bufs=4) as pt_pool,
            tc.tile_pool(name="ot", bufs=3) as ot_pool,
            tc.tile_pool(name="small", bufs=1) as small_pool,
            tc.tile_pool(name="ps_s", bufs=PSS_BUFS, space="PSUM") as ps_s,
            tc.tile_pool(name="ps_proj", bufs=1, space="PSUM") as ps_proj,
            tc.tile_pool(name="ps_o", bufs=3, space="PSUM") as ps_o,
        ):
            wq_t = w_pool.tile([128, KC, GD], f32r, tag="wq")
            wk_t = w_pool.tile([128, KC, GD], f32r, tag="wk")
            wv_t = w_pool.tile([128, KC, GD], f32r, tag="wv")
            # interleave wk and strip-0 x chunks: the first KT projection
            # group consumes them in k order, so it starts after ~2 DMAs
            # instead of waiting behind all the weight loads
            xs0 = xs_pool.tile([128, KC, 512], f32r, tag="xs", name="xs0")
            for k in range(KC):
                nc.sync.dma_start(wk_t[:, k, :],
                                  wkT[k * 128:(k + 1) * 128, :])
                nc.sync.dma_start(xs0[:, k, :],
                                  xT[k * 128:(k + 1) * 128, 0:512])
            for wt, wd in ((wq_t, wqT), (wv_t, wvT)):
                for k in range(KC):
                    nc.sync.dma_start(wt[:, k, :],
                                      wd[k * 128:(k + 1) * 128, :])
            tri = small_pool.tile([128, 128], f32, tag="tri")
            nc.sync.dma_start(tri[:], tri01[:])
            # warm the ACT exp table while input DMAs stream
            warmup = small_pool.tile([1, 1], f32, tag="warmup")
            nc.vector.memset(warmup[:], 0.0)
            nc.scalar.activation(warmup[:], warmup[:], EXP)

            kt = big_pool.tile([128, 4, N], f32r, tag="kt")
            vp = big_pool.tile([128, NB, HPC, HD + 1], f32r, tag="vp")
            # ones column: init whole tile (contiguous memset); V copies
            # overwrite the value columns
            nc.vector.memset(
                vp[:].rearrange("p a b c -> p (a b c)").bitcast(f32), 1.0)

            def load_strip(n):
                xs = xs_pool.tile([128, KC, 512], f32r, tag="xs")
                for k in range(KC):
                    nc.sync.dma_start(
                        xs[:, k, :],
                        xT[k * 128:(k + 1) * 128, n * 512:(n + 1) * 512])
                return xs

            def proj_group(lhs_fn, rhs_fn, copy_out_fn):
                ps = ps_proj.tile([128, 512], f32, tag="psp", name="psp")
                for k in range(KC):
                    nc.tensor.matmul(ps[:], lhs_fn(k), rhs_fn(k),
                                     start=(k == 0), stop=(k == KC - 1))
                copy_out_fn(ps)

            def emit_strip_projections(n, xs, qts):
                """List of closures, one PE-sized projection group each."""
                groups = []
                for m in range(4):      # K^T rows m*128.. for strip n
                    groups.append(lambda m=m: proj_group(
                        lambda k, m=m: wk_t[:, k, m * 128:(m + 1) * 128],
                        lambda k: xs[:, k, :],
                        lambda ps, m=m: nc.vector.tensor_copy(
                            kt[:, m, n * 512:(n + 1) * 512], ps[:]),
                    ))
                for i in range(4):      # V blocks 4n+i
                    mt = 4 * n + i
                    groups.append(lambda mt=mt, i=i: proj_group(
                        lambda k, i=i: xs[:, k, i * 128:(i + 1) * 128],
                        lambda k: wv_t[:, k, :],
                        lambda ps, mt=mt: nc.vector.tensor_copy(
                            vp[:, mt, :, 0:HD],
                            ps[:].rearrange("p (h d) -> p h d", h=HPC)),
                    ))
                for m in range(4):      # Q^T strip n rows m*128..
                    groups.append(lambda m=m: proj_group(
                        lambda k, m=m: wq_t[:, k, m * 128:(m + 1) * 128],
                        lambda k: xs[:, k, :],
                        lambda ps, m=m: nc.vector.tensor_copy(
                            qts[:, m, :], ps[:]),
                    ))
                return groups

            def emit_pv(po, ptc, jj, h, qs, nblocks):
                for idx, j in enumerate(jj):
                    r = j - 4 * qs
                    nstart = 128 * r if r > 0 else 0
                    nc.tensor.matmul(
                        po[:, nstart:512],
                        vp[:, j, h, :],
                        ptc[:, idx, nstart:512],
                        start=(j == 0), stop=(j == nblocks - 1),
                    )

            def attention_pair(h0, qs, qts, weave_fn=None):
                """Heads (h0, h0+1): h0 on PE rows 0-63, h0+1 on rows 64-127.
                QK matmuls interleave the two heads so adjacent MMs hit
                disjoint row groups and stream concurrently."""
                nblocks = 4 * qs + 4
                heads = (h0, h0 + 1)
                m = h0 // 2
                po = {h: ps_o.tile([HD + 1, 512], f32, tag="po",
                                   name=f"po_h{h}")
                      for h in heads}
                pending = {h: None for h in heads}
                for c0 in range(0, nblocks, CH):
                    jj = list(range(c0, min(c0 + CH, nblocks)))
                    w = len(jj)
                    pss = {h: ps_s.tile([128, CH, 512], f32, tag="pss",
                                        name=f"pss_h{h}")
                           for h in heads}
                    for idx, j in enumerate(jj):
                        r = j - 4 * qs
                        nstart = 128 * r if 0 < r < 3 else 0
                        for h in heads:
                            p0 = (h % 2) * 64
                            nc.tensor.matmul(
                                pss[h][:, idx, nstart:512],
                                kt[p0:p0 + 64, m, j * 128:(j + 1) * 128],
                                qts[p0:p0 + 64, m, nstart:512],
                                start=True, stop=True,
                            )
                    ptc = {}
                    for h in heads:
                        ptc[h] = pt_pool.tile([128, CH, 512], f32r,
                                              tag="ptc", name=f"ptc_h{h}")
                        nc.scalar.activation(ptc[h][:, 0:w, :],
                                             pss[h][:, 0:w, :],
                                             EXP, scale=0.125)
                        # causal 0/1 mask on diagonal sub-blocks (SBUF)
                        for idx, j in enumerate(jj):
                            r = j - 4 * qs
                            if r >= 0:
                                nc.vector.tensor_mul(
                                    ptc[h][:, idx, r * 128:(r + 1) * 128],
                                    ptc[h][:, idx, r * 128:(r + 1) * 128],
                                    tri[:],
                                )
                    if weave_fn is not None:
                        weave_fn()
                    for h in heads:
                        if pending[h] is not None:
                            emit_pv(po[h], pending[h][0], pending[h][1],
                                    h, qs, nblocks)
                        pending[h] = (ptc[h], jj)
                for h in heads:
                    emit_pv(po[h], pending[h][0], pending[h][1], h, qs,
                            nblocks)
                    # normalize on device: otile = O^T * bcast(1/sums);
                    # the partition-broadcast runs on GPSIMD (raw sbuf
                    # tensors: the op needs concrete, non-pool APs)
                    i = h % 2
                    nc.vector.reciprocal(rec_raw[i][:],
                                         po[h][HD:HD + 1, :])
                    nc.gpsimd.partition_broadcast(rb_raw[i][:],
                                                  rec_raw[i][:])
                    otile = ot_pool.tile([64, 512], f32, tag="otile",
                                         name=f"otile{h}", bufs=3)
                    nc.vector.tensor_mul(otile[:], po[h][0:HD, :],
                                         rb_raw[i][:])
                    nc.gpsimd.dma_start(
                        outT[h * HD:(h + 1) * HD,
                             qs * 512:(qs + 1) * 512],
                        otile[:],
                    )

            # ---- main emission (self-hosted strips): each strip's
            # projection groups are woven into its OWN attention gaps.
            # Per pair t, only [KT m=t, QT m=t] must precede it; VP groups
            # are woven at 2/chunk during pair 0, ahead of the diagonal
            # PV matmuls that consume them.
            xs = xs0
            for qs in range(NSTRIP):
                qts = qt_pool.tile([128, 4, 512], f32r, tag="qts",
                                   name=f"qts{qs}")
                g = emit_strip_projections(qs, xs, qts)
                # g order: [KT m0..3, VP x4, QT m0..3]
                queue = ([("kq", 0, g[0]), ("kq", 0, g[8])] +
                         [("vp", None, g[4 + i]) for i in range(4)] +
                         [("kq", mm, fn) for mm in (1, 2, 3)
                          for fn in (g[mm], g[8 + mm])])
                # pair-0 requirements upfront
                queue.pop(0)[2]()
                queue.pop(0)[2]()
                if qs + 1 < NSTRIP:
                    xs = load_strip(qs + 1)

                def weave_fn():
                    n = 2 if (queue and queue[0][0] == "vp") else 1
                    for _ in range(n):
                        if queue:
                            queue.pop(0)[2]()

                for t in range(HPC // 2):
                    while queue and any(k == "kq" and mm <= t
                                        for k, mm, _ in queue):
                        queue.pop(0)[2]()
                    attention_pair(2 * t, qs, qts, weave_fn)
                for item in queue:
                    item[2]()

    nc.compile()
    nc.m = get_hw_module(nc.m)
    return nc


_PROGRAM = None


def _program():
    global _PROGRAM
    if _PROGRAM is None:
        _PROGRAM = build_program()
    return _PROGRAM


def make_in_maps(x, Wq, Wk, Wv):
    kk, qq = np.meshgrid(np.arange(128), np.arange(128), indexing="ij")
    tri = (qq >= kk).astype(np.float32)
    in_maps = []
    for c in range(N_CORES):
        b, g = c // 2, c % 2
        sl = slice(g * GD, (g + 1) * GD)
        in_maps.append({
            "xT": np.ascontiguousarray(np.asarray(x)[b].T),
            "wqT": np.ascontiguousarray(np.asarray(Wq).T[:, sl]),
            "wkT": np.ascontiguousarray(np.asarray(Wk).T[:, sl]),
            "wvT": np.ascontiguousarray(np.asarray(Wv).T[:, sl]),
            "tri01": tri,
        })
    return in_maps


def gather(results):
    out = np.empty((B, N, D), np.float32)
    for c in range(N_CORES):
        b, g = c // 2, c % 2
        out[b, :, g * GD:(g + 1) * GD] = results[c]["outT"].T
    return out


def kernel(x, Wq, Wk, Wv):
    nc = _program()
    in_maps = make_in_maps(x, Wq, Wk, Wv)
    res = bass_utils.run_bass_kernel_spmd(nc, in_maps,
                                          core_ids=list(range(N_CORES)))
    return gather(res.results)

